# revision 5
# baseline (speedup 1.0000x reference)
"""Distributed Trainium2 Bass kernel for GQA causal attention with RoPE.

Problem: B=2, S=2048, DIM=2048, 32 Q heads, 8 KV heads (GQA 4:1), hd=64,
causal, rotary embeddings, fp32 in/out (bf16 tensor-engine compute).

Sharding over 8 cores: data-parallel over batch (2) x tensor-parallel over
KV-head groups (4 groups of 2 KV heads, each with its 8 Q heads).
Core c: batch b = c // 4, group g = c % 4.  Each core computes a partial
output projection; the host sums the 4 partials per batch.

Self-contained: only needs /opt/trn_rl_repo (the container's bass stack).
"""
import os
import sys

if "/opt/trn_rl_repo" not in sys.path:
    sys.path.insert(0, "/opt/trn_rl_repo")

import numpy as np

import concourse.bass as bass
import concourse.tile as tile
from concourse import bacc, mybir
from concourse import bass_utils
from concourse.masks import make_identity

F32 = mybir.dt.float32
BF16 = mybir.dt.bfloat16

B, S, D = 2, 2048, 2048
NH, NKV, HD = 32, 8, 64
HL = 8           # local Q heads per core
KVL = 2          # local KV heads per core
EQ = HL * HD     # 512 local q features
EK = KVL * HD    # 128 local k features
EV = KVL * HD    # 128 local v features
EQKV = EQ + EK + EV  # 768
NT = S // 128    # 16 token tiles of 128
NC = S // 512    # 4 token chunks of 512
SCALE = 1.0 / 8.0

# knobs for experiments
XT_MODE = os.environ.get("KERNEL_XT_MODE", "pe")  # "pe" | "dma"

_CACHED_NC = None


def _rope_evict_head(nc, tmp_pool, ps, ps_base, cos, sin, dst, dst_base, dst_cols):
    """Evict one 64-row head from a fp32 psum tile into
    dst[dst_base:dst_base+64, dst_cols] (bf16) applying rotary embedding.

    head_dim is permuted even-first: ps rows ps_base+i hold q[2i] (i<32),
    rows ps_base+32+i hold q[2i+1].
    out rows 0:32  = ps[0:32]*cos - ps[32:64]*sin
    out rows 32:64 = ps[32:64]*cos + ps[0:32]*sin
    """
    p1 = ps[ps_base:ps_base + 32, :]
    p2 = ps[ps_base + 32:ps_base + 64, :]
    t1 = tmp_pool.tile([32, 512], F32, tag="rope_t1")
    nc.vector.tensor_mul(t1[:], p1, cos)
    t2 = tmp_pool.tile([32, 512], F32, tag="rope_t2")
    nc.vector.tensor_mul(t2[:], p2, sin)
    nc.vector.tensor_sub(dst[dst_base:dst_base + 32, dst_cols], t1[:], t2[:])
    t3 = tmp_pool.tile([32, 512], F32, tag="rope_t1")
    nc.vector.tensor_mul(t3[:], p2, cos)
    t4 = tmp_pool.tile([32, 512], F32, tag="rope_t2")
    nc.vector.tensor_mul(t4[:], p1, sin)
    nc.vector.tensor_add(dst[dst_base + 32:dst_base + 64, dst_cols], t3[:], t4[:])


def build():
    nc = bacc.Bacc("TRN2", target_bir_lowering=False, debug=False)
    x_d = nc.dram_tensor("x", [S, D], F32, kind="ExternalInput").ap()
    wq_d = nc.dram_tensor("wq", [D, EQKV], F32, kind="ExternalInput").ap()
    wo_d = nc.dram_tensor("wo", [EQ, D], F32, kind="ExternalInput").ap()
    rope_d = nc.dram_tensor("rope", [64, S], F32, kind="ExternalInput").ap()
    out_d = nc.dram_tensor("out", [S, D], F32, kind="ExternalOutput").ap()

    with tile.TileContext(nc) as tc:
        import contextlib
        ctx = contextlib.ExitStack()
        with ctx:
            const = ctx.enter_context(tc.tile_pool(name="const", bufs=1))
            persist = ctx.enter_context(tc.tile_pool(name="persist", bufs=1))
            stage = ctx.enter_context(tc.tile_pool(name="stage", bufs=2))
            xbp = ctx.enter_context(tc.tile_pool(name="xb", bufs=2))
            xtp = ctx.enter_context(tc.tile_pool(name="xt", bufs=18))
            rope_pool = ctx.enter_context(tc.tile_pool(name="ropep", bufs=4))
            norm_pool = ctx.enter_context(tc.tile_pool(name="normp", bufs=2))
            y_pool = ctx.enter_context(tc.tile_pool(name="yp", bufs=3))
            ppool = ctx.enter_context(tc.tile_pool(name="pp", bufs=6))
            ps_work = ctx.enter_context(
                tc.tile_pool(name="psw", bufs=4, space="PSUM"))
            ps_acc = ctx.enter_context(
                tc.tile_pool(name="psa", bufs=4, space="PSUM"))

            # ---- constants ----
            ident = const.tile([128, 128], BF16)
            make_identity(nc, ident[:])
            cos_t = const.tile([32, S], F32)
            nc.sync.dma_start(cos_t[:], rope_d[0:32, :])
            sin_t = const.tile([32, S], F32)
            nc.sync.dma_start(sin_t[:], rope_d[32:64, :])
            masks = []
            for r in range(4):
                m = const.tile([128, 512], BF16, tag=f"mask{r}")
                nc.gpsimd.memset(m[:], 1.0)
                nc.gpsimd.affine_select(
                    out=m[:], in_=m[:],
                    compare_op=mybir.AluOpType.is_ge,
                    fill=0.0, base=-128 * r, channel_multiplier=-1,
                    pattern=[[1, 512]],
                )
                masks.append(m)

            # ---- weights: load f32, cast to bf16 ----
            wq_bf = []
            for dt in range(16):
                st = stage.tile([128, 2048], F32, tag="xf")
                nc.sync.dma_start(st[:, 0:EQKV], wq_d[128 * dt:128 * (dt + 1), :])
                wb = const.tile([128, EQKV], BF16, tag=f"wqb{dt}")
                nc.vector.tensor_copy(wb[:], st[:, 0:EQKV])
                wq_bf.append(wb)
            wo_bf = []
            for dt in range(4):
                st = stage.tile([128, 2048], F32, tag="xf")
                nc.sync.dma_start(st[:], wo_d[128 * dt:128 * (dt + 1), :])
                wb = const.tile([128, D], BF16, tag=f"wob{dt}")
                nc.vector.tensor_copy(wb[:], st[:])
                wo_bf.append(wb)

            # ---- persistent activation buffers ----
            qT = [persist.tile([128, S], BF16, tag=f"qT{i}", name=f"qT{i}") for i in range(4)]
            kdup = [persist.tile([128, S], BF16, tag=f"kdup{j}", name=f"kdup{j}") for j in range(2)]
            v_aug = [persist.tile([128, 130], BF16, tag=f"vaug{i}", name=f"vaug{i}") for i in range(NT)]
            aoT = [persist.tile([128, S], BF16, tag=f"aoT{i}", name=f"aoT{i}") for i in range(4)]

            for c in range(NC):
                tcol = slice(512 * c, 512 * (c + 1))
                cos_c = cos_t[:, tcol]
                sin_c = sin_t[:, tcol]

                # ---- stage B: x load, cast, transpose ----
                xT = [xtp.tile([128, 512], BF16, tag="xt", name=f"xT_{c}_{i}") for i in range(16)]
                for tt in range(4):
                    trow = slice(512 * c + 128 * tt, 512 * c + 128 * (tt + 1))
                    xf = stage.tile([128, 2048], F32, tag="xf")
                    nc.sync.dma_start(xf[:], x_d[trow, :])
                    xb = xbp.tile([128, 2048], BF16, tag="xb")
                    nc.vector.tensor_copy(xb[:], xf[:])
                    for dt in range(16):
                        pt = ps_work.tile([128, 128], BF16, tag="psw")
                        nc.tensor.transpose(
                            pt[:], xb[:, 128 * dt:128 * (dt + 1)], ident[:])
                        nc.vector.tensor_copy(
                            xT[dt][:, 128 * tt:128 * (tt + 1)], pt[:])

                # ---- QKV matmuls for this chunk ----
                for et in range(5):
                    ps = ps_work.tile([128, 512], F32, tag="psw")
                    for dt in range(16):
                        nc.tensor.matmul(
                            ps[:], wq_bf[dt][:, 128 * et:128 * (et + 1)],
                            xT[dt][:], start=(dt == 0), stop=(dt == 15))
                    if et < 4:
                        for h2 in range(2):
                            _rope_evict_head(
                                nc, rope_pool, ps, 64 * h2, cos_c, sin_c,
                                qT[et], 64 * h2, tcol)
                    else:
                        # k tile: kv head j at rows 64j; duplicate into both
                        # halves of kdup[j]
                        for j in range(2):
                            for half in range(2):
                                _rope_evict_head(
                                    nc, rope_pool, ps, 64 * j, cos_c, sin_c,
                                    kdup[j], 64 * half, tcol)

                # ---- V matmuls (natural layout) ----
                for tt in range(4):
                    it = 4 * c + tt
                    psv = ps_work.tile([128, 128], F32, tag="psw")
                    for dt in range(16):
                        nc.tensor.matmul(
                            psv[:], xT[dt][:, 128 * tt:128 * (tt + 1)],
                            wq_bf[dt][:, 640:768],
                            start=(dt == 0), stop=(dt == 15))
                    nc.vector.tensor_copy(v_aug[it][:, 0:64], psv[:, 0:64])
                    nc.vector.tensor_copy(v_aug[it][:, 65:129], psv[:, 64:128])
                    nc.gpsimd.memset(v_aug[it][:, 64:65], 1.0)
                    nc.gpsimd.memset(v_aug[it][:, 129:130], 1.0)

                # ---- stage C: attention for chunk c ----
                n_tk = 4 * (c + 1)
                for m in range(4):
                    j = m // 2
                    po = [ps_acc.tile([65, 512], F32, tag="psa", name=f"po_{c}_{m}_{i}") for i in range(2)]
                    for kt in range(n_tk):
                        r = kt - 4 * c
                        for h2 in range(2):
                            o = 64 * h2
                            ss = ps_work.tile([128, 512], F32, tag="psw")
                            nc.tensor.matmul(
                                ss[:], kdup[j][o:o + 64, 128 * kt:128 * (kt + 1)],
                                qT[m][o:o + 64, tcol], start=True, stop=True)
                            pbf = ppool.tile([128, 512], BF16, tag="pbf")
                            nc.scalar.activation(
                                pbf[:], ss[:],
                                mybir.ActivationFunctionType.Exp, scale=SCALE)
                            if r >= 0:
                                pm = ppool.tile([128, 512], BF16, tag="pmask")
                                nc.vector.tensor_mul(pm[:], pbf[:], masks[r][:])
                                pbf = pm
                            nc.tensor.matmul(
                                po[h2][:], v_aug[kt][:, 65 * j:65 * j + 65],
                                pbf[:], start=(kt == 0), stop=(kt == n_tk - 1))
                    for h2 in range(2):
                        rc = norm_pool.tile([1, 512], F32, tag="recip")
                        nc.vector.reciprocal(rc[:], po[h2][64:65, :])
                        rb = norm_pool.tile([64, 512], F32, tag="recipb")
                        nc.gpsimd.partition_broadcast(rb[:], rc[:])
                        nc.vector.tensor_mul(
                            aoT[m][64 * h2:64 * h2 + 64, tcol],
                            po[h2][0:64, :], rb[:])

                # ---- stage D: output projection for chunk c ----
                for tt in range(4):
                    trow = slice(512 * c + 128 * tt, 512 * c + 128 * (tt + 1))
                    tcs = slice(512 * c + 128 * tt, 512 * c + 128 * (tt + 1))
                    for ec in range(4):
                        psy = ps_acc.tile([128, 512], F32, tag="psa")
                        for dt in range(4):
                            nc.tensor.matmul(
                                psy[:], aoT[dt][:, tcs],
                                wo_bf[dt][:, 512 * ec:512 * (ec + 1)],
                                start=(dt == 0), stop=(dt == 3))
                        ysb = y_pool.tile([128, 512], F32, tag="ysb")
                        nc.vector.tensor_copy(ysb[:], psy[:])
                        nc.sync.dma_start(
                            out_d[trow, 512 * ec:512 * (ec + 1)], ysb[:])

    nc.compile()
    return nc


def _prep_inputs(x, freqs_cis, wqkv, wo):
    """Host-side sharding: returns list of 8 in_maps."""
    perm = np.concatenate([np.arange(0, HD, 2), np.arange(1, HD, 2)])
    in_maps = []
    for c in range(8):
        b, g = c // 4, c % 4
        wq_rows = wqkv[EQ * g:EQ * (g + 1)].reshape(HL, HD, D)[:, perm, :].reshape(EQ, D)
        wk_rows = wqkv[D + EK * g:D + EK * (g + 1)].reshape(KVL, HD, D)[:, perm, :].reshape(EK, D)
        wv_rows = wqkv[D + NKV * HD + EV * g:D + NKV * HD + EV * (g + 1)]
        wq_cat = np.concatenate([wq_rows, wk_rows, wv_rows], axis=0)
        rope = np.concatenate(
            [freqs_cis[:, :, 0].T, freqs_cis[:, :, 1].T], axis=0)
        in_maps.append({
            "x": np.ascontiguousarray(x[b]),
            "wq": np.ascontiguousarray(wq_cat.T),
            "wo": np.ascontiguousarray(wo[:, EQ * g:EQ * (g + 1)].T),
            "rope": np.ascontiguousarray(rope.astype(np.float32)),
        })
    return in_maps


def _get_nc():
    global _CACHED_NC
    if _CACHED_NC is None:
        _CACHED_NC = build()
    return _CACHED_NC


def kernel(x, freqs_cis, wqkv, wo, _trace=False, _trace_kwargs=None):
    nc = _get_nc()
    in_maps = _prep_inputs(x, freqs_cis, wqkv, wo)
    res = bass_utils.run_bass_kernel_spmd(
        nc, in_maps, core_ids=list(range(8)), trace=_trace,
        **(_trace_kwargs or {}))
    outs = [res.results[c]["out"] for c in range(8)]
    y = np.stack([
        outs[0] + outs[1] + outs[2] + outs[3],
        outs[4] + outs[5] + outs[6] + outs[7],
    ]).astype(np.float32)
    kernel.last_results = res
    return y


# revision 19
# speedup vs baseline: 1.3575x; 1.3575x over previous
"""Distributed Trainium2 Bass kernel for GQA causal attention with RoPE.

Problem: B=2, S=2048, DIM=2048, 32 Q heads, 8 KV heads (GQA 4:1), hd=64,
causal, rotary embeddings, fp32 in/out (bf16 tensor-engine compute).

Sharding over 8 cores: data-parallel over batch (2) x tensor-parallel over
KV-head groups (4 groups of 2 KV heads, each with its 8 Q heads).
Core c: batch b = c // 4, group g = c % 4.  Each core computes a partial
output projection; the host sums the 4 partials per batch.

Self-contained: only needs /opt/trn_rl_repo (the container's bass stack).
"""
import os
import sys

if "/opt/trn_rl_repo" not in sys.path:
    sys.path.insert(0, "/opt/trn_rl_repo")

import contextlib

import numpy as np

import concourse.bass as bass
import concourse.tile as tile
from concourse import bacc, mybir
from concourse import bass_utils

F32 = mybir.dt.float32
BF16 = mybir.dt.bfloat16
EXP = mybir.ActivationFunctionType.Exp

B, S, D = 2, 2048, 2048
NH, NKV, HD = 32, 8, 64
HL = 8           # local Q heads per core
KVL = 2          # local KV heads per core
EQ = HL * HD     # 512 local q features
EK = KVL * HD    # 128
EV = KVL * HD    # 128
EQKV = EQ + EK + EV  # 768
NT = S // 128    # 16 token tiles
NC = S // 512    # 4 token chunks
SCALE = 1.0 / 8.0

_CACHED_NC = None


def _rope_tile(nc, tp, ps, cosF, sinF, dst, dst_cols):
    """RoPE on a full [128, 512] psum tile (2 heads, head_dim permuted
    even-first within each 64-row head) -> dst[0:128, dst_cols] (bf16).

    For each head block at base o in {0, 64}:
      out[o:o+32]    = p[o:o+32]*cos    - p[o+32:o+64]*sin
      out[o+32:o+64] = p[o+32:o+64]*cos + p[o:o+32]*sin
    Implemented as: t1 = swap(p) * sinF (sinF rows: -sin,+sin),
    t2 = p * cosF, out = t2 + t1  (6 DVE ops per tile).
    """
    t1 = tp.tile([128, 512], F32, tag="rope_t1")
    for o in (0, 64):
        nc.vector.tensor_mul(t1[o:o + 32, :], ps[o + 32:o + 64, :],
                             sinF[0:32, :])
        nc.vector.tensor_mul(t1[o + 32:o + 64, :], ps[o:o + 32, :],
                             sinF[32:64, :])
    t2 = tp.tile([128, 512], F32, tag="rope_t2")
    nc.vector.tensor_mul(t2[:], ps[:], cosF[:])
    nc.vector.tensor_add(dst[:, dst_cols], t2[:], t1[:])


def _rope_kdup(nc, tp, ps, j, cosF, sinF, dst, dst_cols):
    """RoPE one kv head (psum rows 64j:64j+64) duplicated into both halves
    of dst (a kdup tile) at dst_cols."""
    o = 64 * j
    t1 = tp.tile([64, 512], F32, tag="rope_kt1")
    nc.vector.tensor_mul(t1[0:32, :], ps[o + 32:o + 64, :], sinF[0:32, :])
    nc.vector.tensor_mul(t1[32:64, :], ps[o:o + 32, :], sinF[32:64, :])
    t2 = tp.tile([64, 512], F32, tag="rope_kt2")
    nc.vector.tensor_mul(t2[:], ps[o:o + 64, :], cosF[0:64, :])
    nc.vector.tensor_add(dst[0:64, dst_cols], t2[:], t1[:])
    nc.vector.tensor_add(dst[64:128, dst_cols], t2[:], t1[:])


def build():
    nc = bacc.Bacc("TRN2", target_bir_lowering=False, debug=False)
    x_d = nc.dram_tensor("x", [D, S], F32, kind="ExternalInput").ap()
    wq_d = nc.dram_tensor("wq", [D, EQKV], F32, kind="ExternalInput").ap()
    wo_d = nc.dram_tensor("wo", [EQ, D], F32, kind="ExternalInput").ap()
    # rope rows: 0:128 = cos x4, 128:192 = [-sin, +sin]
    rope_d = nc.dram_tensor("rope", [192, S], F32, kind="ExternalInput").ap()
    out_d = nc.dram_tensor("out", [S, D], F32, kind="ExternalOutput").ap()
    DEBUG = bool(int(os.environ.get("KDEBUG", "0")))
    dbg = {}
    if DEBUG:
        for nm, shp in [("d_xT0", [128, 512]), ("d_qT0", [128, S]),
                        ("d_kdup0", [128, S]), ("d_vaug0", [128, 130]),
                        ("d_pbf", [128, 512]), ("d_dn", [128, 512]),
                        ("d_aoT0", [128, S])]:
            dbg[nm] = nc.dram_tensor(nm, shp, BF16, kind="ExternalOutput").ap()

    with tile.TileContext(nc) as tc:
        ctx = contextlib.ExitStack()
        with ctx:
            const = ctx.enter_context(tc.tile_pool(name="const", bufs=1))
            persist = ctx.enter_context(tc.tile_pool(name="persist", bufs=1))
            stage = ctx.enter_context(tc.tile_pool(name="stage", bufs=2))
            xtp = ctx.enter_context(tc.tile_pool(name="xt", bufs=17))
            ropep = ctx.enter_context(tc.tile_pool(name="ropep", bufs=2))
            osbp = ctx.enter_context(tc.tile_pool(name="osb", bufs=9))
            normp = ctx.enter_context(tc.tile_pool(name="normp", bufs=1))
            rbp = ctx.enter_context(tc.tile_pool(name="rbp", bufs=3))
            y_pool = ctx.enter_context(tc.tile_pool(name="yp", bufs=3))
            ppool = ctx.enter_context(tc.tile_pool(name="pp", bufs=6))
            ps_sc = ctx.enter_context(
                tc.tile_pool(name="pssc", bufs=4, space="PSUM"))
            ps_misc = ctx.enter_context(
                tc.tile_pool(name="psmc", bufs=2, space="PSUM"))
            ps_acc = ctx.enter_context(
                tc.tile_pool(name="psac", bufs=2, space="PSUM"))

            # ---- constants ----
            cosF = const.tile([128, S], F32)
            nc.sync.dma_start(cosF[:], rope_d[0:128, :])
            sinF = const.tile([64, S], F32)
            nc.sync.dma_start(sinF[:], rope_d[128:192, :])

            # ---- weights: f32 loads + ScalarE cast to bf16 ----
            COPYF = mybir.ActivationFunctionType.Copy
            wq_bf = []
            for dt in range(16):
                st = stage.tile([128, 2048], F32, tag="xf", name=f"wst{dt}")
                nc.sync.dma_start(st[:, 0:EQKV], wq_d[128 * dt:128 * (dt + 1), :])
                wb = const.tile([128, EQKV], BF16, tag=f"wqb{dt}",
                                name=f"wqb{dt}")
                nc.scalar.activation(wb[:], st[:, 0:EQKV], COPYF)
                wq_bf.append(wb)
            wo_bf = []
            for dt in range(4):
                st = stage.tile([128, 2048], F32, tag="xf", name=f"wost{dt}")
                nc.sync.dma_start(st[:], wo_d[128 * dt:128 * (dt + 1), :])
                wb = const.tile([128, D], BF16, tag=f"wob{dt}", name=f"wob{dt}")
                nc.scalar.activation(wb[:], st[:], COPYF)
                wo_bf.append(wb)


            # ---- persistent activation buffers ----
            qT = [persist.tile([128, S], BF16, tag=f"qT{i}", name=f"qT{i}")
                  for i in range(4)]
            kdup = [persist.tile([128, S], BF16, tag=f"kdup{j}", name=f"kdup{j}")
                    for j in range(2)]
            v_aug = [persist.tile([128, 130], BF16, tag=f"vaug{i}",
                                  name=f"vaug{i}") for i in range(NT)]
            aoT = [persist.tile([128, S], BF16, tag=f"aoT{i}", name=f"aoT{i}")
                   for i in range(4)]

            for c in range(NC):
                tcol = slice(512 * c, 512 * (c + 1))
                cos_c = cosF[:, tcol]
                sin_c = sinF[:, tcol]

                # ---- stage B: load host-transposed x panels, cast bf16 ----
                xT = [xtp.tile([128, 512], BF16, tag="xt", name=f"xT_{c}_{i}")
                      for i in range(16)]
                for dt in range(16):
                    xs = stage.tile([128, 512], F32, tag="xs")
                    nc.sync.dma_start(
                        xs[:], x_d[128 * dt:128 * (dt + 1), tcol])
                    nc.gpsimd.tensor_copy(xT[dt][:], xs[:])
                if DEBUG and c == 0:
                    nc.sync.dma_start(dbg["d_xT0"][:], xT[0][:])

                # ---- QKV matmuls ----
                for et in range(5):
                    ps = ps_misc.tile([128, 512], F32, tag="psmc")
                    for dt in range(16):
                        nc.tensor.matmul(
                            ps[:], wq_bf[dt][:, 128 * et:128 * (et + 1)],
                            xT[dt][:], start=(dt == 0), stop=(dt == 15))
                    if et < 4:
                        _rope_tile(nc, ropep, ps, cos_c, sin_c, qT[et], tcol)
                    else:
                        for j in range(2):
                            _rope_kdup(nc, ropep, ps, j, cos_c, sin_c,
                                       kdup[j], tcol)

                # ---- V matmuls (natural layout) + ones column ----
                for tt in range(4):
                    it = 4 * c + tt
                    psv = ps_misc.tile([128, 128], F32, tag="psmc")
                    for dt in range(16):
                        nc.tensor.matmul(
                            psv[:], xT[dt][:, 128 * tt:128 * (tt + 1)],
                            wq_bf[dt][:, 640:768],
                            start=(dt == 0), stop=(dt == 15))
                    nc.vector.tensor_copy(v_aug[it][:, 0:64], psv[:, 0:64])
                    nc.vector.tensor_copy(v_aug[it][:, 65:129], psv[:, 64:128])
                    nc.gpsimd.memset(v_aug[it][:, 64:65], 1.0)
                    nc.gpsimd.memset(v_aug[it][:, 129:130], 1.0)

                if DEBUG and c == 0:
                    nc.sync.dma_start(dbg["d_qT0"][:], qT[0][:])
                    nc.sync.dma_start(dbg["d_kdup0"][:], kdup[0][:])
                    nc.sync.dma_start(dbg["d_vaug0"][:], v_aug[0][:])
                # ---- stage C: attention ----
                n_tk = 4 * (c + 1)
                # denominators land at 32-aligned rows of two [128,512] tiles
                dn = [normp.tile([128, 512], F32, tag=f"dn{i}", name=f"dn{c}_{i}")
                      for i in range(2)]
                for i in range(2):
                    nc.gpsimd.memset(dn[i][:], 1.0)
                o_sb = []
                for m in range(4):
                    j = m // 2
                    po = [ps_acc.tile([65, 512], F32, tag="psac",
                                      name=f"po_{c}_{m}_{i}") for i in range(2)]
                    for kt in range(n_tk):
                        r = kt - 4 * c
                        for h2 in range(2):
                            o = 64 * h2
                            ss = ps_sc.tile([128, 512], F32, tag="pssc")
                            nc.tensor.matmul(
                                ss[:],
                                kdup[j][o:o + 64, 128 * kt:128 * (kt + 1)],
                                qT[m][o:o + 64, tcol], start=True, stop=True)
                            pbf = ppool.tile([128, 512], BF16, tag="pbf")
                            nc.scalar.activation(pbf[:], ss[:], EXP,
                                                 scale=SCALE)
                            if r >= 0:
                                pm = ppool.tile([128, 512], BF16, tag="pmask")
                                nc.gpsimd.affine_select(
                                    out=pm[:], in_=pbf[:],
                                    compare_op=mybir.AluOpType.is_ge,
                                    fill=0.0, base=-128 * r,
                                    channel_multiplier=-1,
                                    pattern=[[1, 512]])
                                pbf = pm
                            if DEBUG and c == 0 and m == 0 and kt == 0 and h2 == 0:
                                nc.sync.dma_start(dbg["d_pbf"][:], pbf[:])
                            nc.tensor.matmul(
                                po[h2][:], v_aug[kt][:, 65 * j:65 * j + 65],
                                pbf[:], start=(kt == 0), stop=(kt == n_tk - 1))
                    for h2 in range(2):
                        i8 = 2 * m + h2
                        ob = osbp.tile([64, 512], BF16, tag="ob",
                                       name=f"ob_{c}_{m}_{h2}")
                        nc.vector.tensor_copy(ob[:], po[h2][0:64, :])
                        row = 32 * (i8 % 4)
                        nc.vector.tensor_copy(
                            dn[i8 // 4][row:row + 1, :], po[h2][64:65, :])
                        o_sb.append(ob)
                # batched reciprocal of all 8 denominators for this chunk
                dnrb = []
                for i in range(2):
                    dnr = normp.tile([128, 512], F32, tag=f"dnr{i}",
                                     name=f"dnr{c}_{i}")
                    nc.vector.reciprocal(dnr[:], dn[i][:])
                    db = normp.tile([128, 512], BF16, tag=f"dnrb{i}",
                                    name=f"dnrb{c}_{i}")
                    nc.vector.tensor_copy(db[:], dnr[:])
                    dnrb.append(db)
                if DEBUG and c == 0:
                    nc.sync.dma_start(dbg["d_dn"][:], dnrb[0][:])
                for m in range(4):
                    for h2 in range(2):
                        i8 = 2 * m + h2
                        row = 32 * (i8 % 4)
                        r1 = rbp.tile([1, 512], BF16, tag="r1")
                        nc.vector.tensor_copy(
                            r1[:], dnrb[i8 // 4][row:row + 1, :])
                        rb = rbp.tile([64, 512], BF16, tag="rb")
                        nc.gpsimd.partition_broadcast(rb[:], r1[:])
                        nc.vector.tensor_mul(
                            aoT[m][64 * h2:64 * h2 + 64, tcol],
                            o_sb[i8][0:64, :], rb[:])

                if DEBUG and c == 3:
                    nc.sync.dma_start(dbg["d_aoT0"][:], aoT[0][:])
                # ---- stage D: output projection ----
                for tt in range(4):
                    trow = slice(512 * c + 128 * tt, 512 * c + 128 * (tt + 1))
                    for ec in range(4):
                        psy = ps_sc.tile([128, 512], F32, tag="pssc")
                        for dt in range(4):
                            nc.tensor.matmul(
                                psy[:], aoT[dt][:, trow],
                                wo_bf[dt][:, 512 * ec:512 * (ec + 1)],
                                start=(dt == 0), stop=(dt == 3))
                        ysb = y_pool.tile([128, 512], F32, tag="ysb")
                        nc.vector.tensor_copy(ysb[:], psy[:])
                        nc.sync.dma_start(
                            out_d[trow, 512 * ec:512 * (ec + 1)], ysb[:])

    nc.compile()
    return nc


def _prep_inputs(x, freqs_cis, wqkv, wo):
    """Host-side sharding: returns list of 8 in_maps."""
    perm = np.concatenate([np.arange(0, HD, 2), np.arange(1, HD, 2)])
    cos = np.ascontiguousarray(freqs_cis[:, :, 0].T.astype(np.float32))  # [32,S]
    sin = np.ascontiguousarray(freqs_cis[:, :, 1].T.astype(np.float32))
    rope = np.ascontiguousarray(
        np.concatenate([cos, cos, cos, cos, -sin, sin], axis=0))  # [192,S]
    in_maps = []
    xT_by_b = [np.ascontiguousarray(x[b].T) for b in range(B)]
    for c in range(8):
        b, g = c // 4, c % 4
        wq_rows = wqkv[EQ * g:EQ * (g + 1)].reshape(HL, HD, D)[:, perm, :].reshape(EQ, D)
        wk_rows = wqkv[D + EK * g:D + EK * (g + 1)].reshape(KVL, HD, D)[:, perm, :].reshape(EK, D)
        wv_rows = wqkv[D + NKV * HD + EV * g:D + NKV * HD + EV * (g + 1)]
        wq_cat = np.concatenate([wq_rows, wk_rows, wv_rows], axis=0)
        in_maps.append({
            "x": xT_by_b[b],
            "wq": np.ascontiguousarray(wq_cat.T),
            "wo": np.ascontiguousarray(wo[:, EQ * g:EQ * (g + 1)].T),
            "rope": rope,
        })
    return in_maps


def _get_nc():
    global _CACHED_NC
    if _CACHED_NC is None:
        _CACHED_NC = build()
    return _CACHED_NC


def kernel(x, freqs_cis, wqkv, wo, _trace=False, _trace_kwargs=None):
    nc = _get_nc()
    in_maps = _prep_inputs(x, freqs_cis, wqkv, wo)
    res = bass_utils.run_bass_kernel_spmd(
        nc, in_maps, core_ids=list(range(8)), trace=_trace,
        **(_trace_kwargs or {}))
    outs = [res.results[c]["out"] for c in range(8)]
    y = np.stack([
        outs[0] + outs[1] + outs[2] + outs[3],
        outs[4] + outs[5] + outs[6] + outs[7],
    ]).astype(np.float32)
    kernel.last_results = res
    return y
